# revision 14
# baseline (speedup 1.0000x reference)
"""Trainium2 Bass kernel for MambaMomentum (B=1, L=2048, D=1024, ED=2048, N=16).

Tensor-parallel over d_inner (ED) across 8 NeuronCores; each core owns 256
channels end-to-end. The one cross-core dependency (dBC = xc @ W_x.T, a
full-ED contraction) is handled by splitting the kernel into two launches
with a host-side 8-way sum of the small (96 x 2048) partials between them —
the on-device AllReduce costs ~80us of latency-floor, the host reduce is
free.

Launch A: in_proj (f32r matmuls), depthwise causal conv, SiLU, x_proj
partials. Launch B: dt_proj/softplus, the (ED x N) selective scan with
momentum (DVE TensorTensorScan in bf16, channels on partitions, time on the
free dim), state reduction over N via PE identity-matmul accumulation in
PSUM, gating, out_proj partials (summed on host).
"""

import sys

if "/opt/trn_rl_repo" not in sys.path:
    sys.path.insert(0, "/opt/trn_rl_repo")

import numpy as np
import ml_dtypes

import concourse.bass as bass
import concourse.mybir as mybir
from concourse.tile import TileContext

N_CORES = 8
D_MODEL = 1024
ED = 2048
N_ST = 16
DT_RANK = 64
K_CONV = 4
BETA = 0.6
ALPHA = 1.0
L = 2048
E = ED // N_CORES  # 256
NE = E // 128      # 2
NT = L // 512      # 4
DBC = DT_RANK + 2 * N_ST  # 96
BF16 = mybir.dt.bfloat16
F32 = mybir.dt.float32
F32R = mybir.dt.float32r
AF = mybir.ActivationFunctionType
OP = mybir.AluOpType

_CACHE = {}


def _split_ctrl_waits(nc, max_waits=1):
    """walrus CoreV3 codegen rejects >1 sem-wait on several encodings; move
    excess waits onto single-wait NoOps inserted just before."""
    for fn in nc.m.functions:
        for bb in fn.blocks:
            new_insts = []
            for inst in bb.instructions:
                si = inst.sync_info
                if si is not None and si.on_wait and len(si.on_wait) > max_waits:
                    waits = list(si.on_wait)
                    si.on_wait = waits[:max_waits]
                    extra = waits[max_waits:]
                    for i in range(0, len(extra), max_waits):
                        new_insts.append(mybir.InstNoOp(
                            name=f"{inst.name}_ws{i}",
                            engine=inst.engine,
                            ins=[], outs=[],
                            sync_info=mybir.SyncInfo(
                                on_wait=extra[i:i + max_waits], on_update=[]),
                        ))
                new_insts.append(inst)
            bb.instructions[:] = new_insts


def _build_a():
    nc = bass.Bass("TRN2", target_bir_lowering=False, debug=False,
                   num_devices=N_CORES)
    xT = nc.dram_tensor("xT", [D_MODEL, L], F32R, kind="ExternalInput")
    wxcT = nc.dram_tensor("wxcT", [D_MODEL, E], F32R, kind="ExternalInput")
    convw = nc.dram_tensor("convw", [E, K_CONV], F32, kind="ExternalInput")
    convb = nc.dram_tensor("convb", [E, 1], F32, kind="ExternalInput")
    wxT = nc.dram_tensor("wxT", [E, DBC], F32R, kind="ExternalInput")
    xc_o = nc.dram_tensor("xc_o", [E, L], F32R, kind="ExternalOutput")
    dbcp_o = nc.dram_tensor("dbcp_o", [DBC, L], BF16, kind="ExternalOutput")

    with TileContext(nc) as tc:
        with (
            tc.tile_pool(name="prm", bufs=1) as prm,
            tc.tile_pool(name="xin", bufs=1) as xin,
            tc.tile_pool(name="wts", bufs=1) as wts,
            tc.tile_pool(name="stg", bufs=2) as stg,
            tc.tile_pool(name="stg1", bufs=1) as stg1,
            tc.tile_pool(name="psA", bufs=1, space="PSUM") as psA,
        ):
            w_in_t = wts.tile([128, 8, E], F32R, tag="w_in")
            x_t = xin.tile([128, 8, L], F32R, tag="x")
            for k in range(8):
                ksl = slice(k * 128, (k + 1) * 128)
                nc.sync.dma_start(out=w_in_t[:, k, :], in_=wxcT[ksl, :])
                nc.sync.dma_start(out=x_t[:, k, :], in_=xT[ksl, :])
            convw_t = prm.tile([128, NE, K_CONV], F32, tag="convw")
            convb_t = prm.tile([128, NE, 1], F32, tag="convb")
            wx_t = prm.tile([128, NE, DBC], F32R, tag="wx")
            for m in range(NE):
                sl = slice(m * 128, (m + 1) * 128)
                nc.gpsimd.dma_start(out=convw_t[:, m, :], in_=convw[sl, :])
                nc.gpsimd.dma_start(out=convb_t[:, m, :], in_=convb[sl, :])
                nc.gpsimd.dma_start(out=wx_t[:, m, :], in_=wxT[sl, :])

            # PE warm-up: ~4us of junk matmuls so in_proj runs at 2.4 GHz
            wu_ps = psA.tile([128, 512], F32, tag="pA00", name="warm_ps")
            for _w in range(20):
                nc.tensor.matmul(wu_ps[:], w_in_t[:, 0, 0:128],
                                 x_t[:, 0, 0:512], start=True, stop=True)

            xc_t = [None] * NE
            for m in range(NE):
                psx = [psA.tile([128, 512], F32, tag=f"pA{m}{t}",
                                name=f"psx{m}{t}") for t in range(NT)]
                for k in range(8):
                    for t in range(NT):
                        nc.tensor.matmul(psx[t][:],
                                         w_in_t[:, k, m * 128:(m + 1) * 128],
                                         x_t[:, k, t * 512:(t + 1) * 512],
                                         start=(k == 0), stop=(k == 7))
                raw = stg.tile([128, L], F32, tag="xcraw")
                for t in range(NT):
                    nc.scalar.copy(raw[:, t * 512:(t + 1) * 512], psx[t][:])
                acc = stg1.tile([128, L], F32, tag="convacc")
                cw = convw_t[:, m, :]
                nc.vector.tensor_scalar_mul(acc[:, :], raw[:, :], cw[:, 3:4])
                for kk in range(1, K_CONV):
                    nc.vector.scalar_tensor_tensor(
                        acc[:, kk:], raw[:, :L - kk], cw[:, 3 - kk:4 - kk],
                        acc[:, kk:], OP.mult, OP.add)
                xc_t[m] = stg1.tile([128, L], F32R, tag=f"xc{m}",
                                    name=f"xc_t{m}")
                nc.scalar.activation(xc_t[m][:, :], acc[:, :], AF.Silu,
                                     bias=convb_t[:, m, :], scale=1.0)
                nc.sync.dma_start(out=xc_o[m * 128:(m + 1) * 128, :],
                                  in_=xc_t[m][:, :])

            # x_proj partial
            for t in range(NT):
                ps = psA.tile([128, 512], F32, tag=f"pA0{t}", name=f"psb{t}")
                for m in range(NE):
                    nc.tensor.matmul(ps[0:DBC, :], wx_t[:, m, :],
                                     xc_t[m][:, t * 512:(t + 1) * 512],
                                     start=(m == 0), stop=(m == NE - 1))
                dst = stg.tile([DBC, 512], BF16, tag="dbcp")
                nc.scalar.copy(dst[:, :], ps[0:DBC, :])
                nc.sync.dma_start(out=dbcp_o[:, t * 512:(t + 1) * 512],
                                  in_=dst[:, :])

    _split_ctrl_waits(nc)
    return nc


def _build_b():
    from concourse.bass import _add_dep_helper

    nc = bass.Bass("TRN2", target_bir_lowering=False, debug=False,
                   num_devices=N_CORES)
    xc_i = nc.dram_tensor("xc_i", [E, L], F32R, kind="ExternalInput")
    xT = nc.dram_tensor("xT", [D_MODEL, L], F32R, kind="ExternalInput")
    wzT = nc.dram_tensor("wzT", [D_MODEL, E], F32R, kind="ExternalInput")
    dbc_i = nc.dram_tensor("dbc_i", [DBC, L], BF16, kind="ExternalInput")
    wdtT = nc.dram_tensor("wdtT", [DT_RANK, E], BF16, kind="ExternalInput")
    bdt = nc.dram_tensor("bdt", [E, 1], F32, kind="ExternalInput")
    acols = nc.dram_tensor("acols", [E, N_ST], F32, kind="ExternalInput")
    dcol = nc.dram_tensor("dcol", [E, 1], F32, kind="ExternalInput")
    woutT = nc.dram_tensor("woutT", [E, D_MODEL], BF16, kind="ExternalInput")
    ident = nc.dram_tensor("ident", [128, 128], BF16, kind="ExternalInput")
    w0fir = nc.dram_tensor("w0fir", [128, 128], BF16, kind="ExternalInput")
    w1fir = nc.dram_tensor("w1fir", [128, 128], BF16, kind="ExternalInput")
    out_pT = nc.dram_tensor("out_pT", [D_MODEL, L], F32, kind="ExternalOutput")
    dbc_ap = dbc_i.ap()
    NB = L // 128  # 16 time blocks of 128
    NG = 4         # n-group size for batched transpose / FIR

    def ebl(t3, m):
        return t3[:, m, :]

    # XBAR transposes corrupt each other when concurrent; chain-serialize all.
    tchain = []

    def xpose(out, in_):
        t = nc.sync.dma_start_transpose(out=out, in_=in_)
        if tchain:
            _add_dep_helper(t.ins, tchain[-1].ins, sync=True,
                            reason="xbar serialize")
        tchain.append(t)
        return t

    with TileContext(nc) as tc:
        with (
            tc.tile_pool(name="res", bufs=1) as res,
            tc.tile_pool(name="prm", bufs=1) as prm,
        ):
            xc_t = res.tile([128, NE, L], BF16, tag="xc")
            zs_t = res.tile([128, NE, L], BF16, tag="zs")
            delta_t = res.tile([128, NE, L], F32, tag="delta")
            wu_t = res.tile([128, NE, L], BF16, tag="wu")
            wout_t = res.tile([128, NE, D_MODEL], BF16, tag="wout")

            bdt_t = prm.tile([128, NE, 1], F32, tag="bdt")
            acols_t = prm.tile([128, NE, N_ST], F32, tag="acols")
            dcol_t = prm.tile([128, NE, 1], F32, tag="dcol")
            wdt_t = prm.tile([DT_RANK, E], BF16, tag="wdt")
            ident_t = prm.tile([128, 128], BF16, tag="ident")
            w0_t = prm.tile([128, 128], BF16, tag="w0")
            w1_t = prm.tile([128, 128], BF16, tag="w1")
            dbcd_t = prm.tile([DT_RANK, L], BF16, tag="dbcd")

            # order matters: the delta-chain inputs first
            nc.gpsimd.dma_start(out=dbcd_t[:, :], in_=dbc_i[0:DT_RANK, :])
            nc.gpsimd.dma_start(out=wdt_t[:, :], in_=wdtT[:, :])
            nc.gpsimd.dma_start(out=ident_t[:, :], in_=ident[:, :])
            nc.gpsimd.dma_start(out=w0_t[:, :], in_=w0fir[:, :])
            nc.gpsimd.dma_start(out=w1_t[:, :], in_=w1fir[:, :])
            for m in range(NE):
                sl = slice(m * 128, (m + 1) * 128)
                nc.gpsimd.dma_start(out=bdt_t[:, m, :], in_=bdt[sl, :])
                nc.gpsimd.dma_start(out=acols_t[:, m, :], in_=acols[sl, :])
                nc.gpsimd.dma_start(out=dcol_t[:, m, :], in_=dcol[sl, :])
            for m in range(NE):
                sl = slice(m * 128, (m + 1) * 128)
                nc.scalar.dma_start(out=wout_t[:, m, :], in_=woutT[sl, :])

            with (
                tc.tile_pool(name="stg2", bufs=2) as stg2,
                tc.tile_pool(name="xcl", bufs=1) as xcl,
                tc.tile_pool(name="psD", bufs=4, space="PSUM") as psD,
            ):
                xcf = xcl.tile([128, NE, L], F32R, tag="xcf")
                for m in range(NE):
                    sl = slice(m * 128, (m + 1) * 128)
                    nc.scalar.dma_start(out=ebl(xcf, m), in_=xc_i[sl, :])
                    nc.scalar.copy(ebl(xc_t, m), ebl(xcf, m).bitcast(F32))
                warmact = stg2.tile([128, 1], F32, tag="warmact")
                nc.scalar.activation(warmact[:, :], bdt_t[:, 0, :], AF.Exp)
                for m in range(NE):
                    dd = ebl(delta_t, m)
                    for t in range(NT):
                        ps = psD.tile([128, 512], F32, tag="pD")
                        nc.tensor.matmul(ps[:], wdt_t[:, m * 128:(m + 1) * 128],
                                         dbcd_t[:, t * 512:(t + 1) * 512],
                                         start=True, stop=True)
                        # softplus(x+b) = Ln(1+Exp(x+b)); x+b in [-9.3,-2.2]
                        nc.scalar.activation(dd[:, t * 512:(t + 1) * 512], ps[:],
                                             AF.Exp, bias=bdt_t[:, m, :], scale=1.0)
                    nc.vector.tensor_scalar_add(dd, dd, 1.0)
                    nc.scalar.activation(dd, dd, AF.Ln)
                    nc.vector.tensor_tensor(out=ebl(wu_t, m), in0=dd,
                                            in1=ebl(xcf, m).bitcast(F32),
                                            op=OP.mult)

            # ---- z half of in_proj (own psum scope, silu fused from psum) ----
            with (
                tc.tile_pool(name="pbz", bufs=1) as pbz,
                tc.tile_pool(name="repz", bufs=2) as repz,
                tc.tile_pool(name="psZ", bufs=1, space="PSUM") as psZ,
            ):
                z_ps = [psZ.tile([128, L], F32, tag=f"z{m}", name=f"z_ps{m}")
                        for m in range(NE)]
                wz_t = pbz.tile([128, 8, E], F32R, tag="wz")
                for k in range(8):
                    nc.gpsimd.dma_start(out=wz_t[:, k, :],
                                        in_=wzT[k * 128:(k + 1) * 128, :])
                for k in range(8):
                    xbk = repz.tile([128, L], F32R, tag="xbk")
                    nc.gpsimd.dma_start(out=xbk[:, :],
                                        in_=xT[k * 128:(k + 1) * 128, :])
                    for m in range(NE):
                        for t in range(NT):
                            nc.tensor.matmul(
                                z_ps[m][:, t * 512:(t + 1) * 512],
                                wz_t[:, k, m * 128:(m + 1) * 128],
                                xbk[:, t * 512:(t + 1) * 512],
                                start=(k == 0), stop=(k == 7))
                for m in range(NE):
                    for t in range(NT):
                        nc.scalar.activation(
                            ebl(zs_t, m)[:, t * 512:(t + 1) * 512],
                            z_ps[m][:, t * 512:(t + 1) * 512], AF.Silu)

            # =================== momentum-FIR + scan ===================
            # v = EMA_beta(u) == 16-tap FIR (err beta^16 ~ 2.8e-4), computed on
            # PE in time-major: u -> xbar-transpose -> Toeplitz matmuls -> v_T
            # -> transpose back. DVE only runs the 32 h-scans.
            with (
                tc.tile_pool(name="fir", bufs=1) as fir,
                tc.tile_pool(name="rep", bufs=2) as rep,
                tc.tile_pool(name="sc", bufs=2) as sc,
                tc.tile_pool(name="psF", bufs=2, space="PSUM") as psF,
                tc.tile_pool(name="psY", bufs=1, space="PSUM") as psY,
            ):
                for m in range(NE):
                    y_ps = psY.tile([128, L], F32, tag="y", name=f"y_ps{m}")
                    for g in range(N_ST // NG):
                        # ---- u = wu * bm for NG states, grouped ----
                        ug = fir.tile([128, NG, L], BF16, tag="ug")
                        for j in range(NG):
                            n = g * NG + j
                            bm_rep = rep.tile([128, L], BF16, tag="bm", bufs=4)
                            nc.gpsimd.dma_start(
                                out=bm_rep[:, :],
                                in_=bass.AP(tensor=dbc_ap.tensor,
                                            offset=(DT_RANK + n) * L,
                                            ap=[[0, 128], [1, L]]))
                            nc.vector.tensor_tensor(out=ug[:, j, :],
                                                    in0=ebl(wu_t, m),
                                                    in1=bm_rep[:, :],
                                                    op=OP.mult)
                        # ---- batched transpose: [128e, NG*L] -> [128t, B, 128e]
                        # B = j*NB + b  (b = time block)
                        utg = fir.tile([128, NG * NB, 128], BF16, tag="utg")
                        xpose(utg[:, :, :], ug[:, :, :])
                        # ---- PE FIR into psum, 2 time blocks per psum tile ----
                        # psum + vtg free order: [j][b][e] so each n's v is
                        # contiguous after the reverse transpose
                        vtg = fir.tile([128, NG, NB, 128], BF16, tag="vtg")
                        for bp in range(NB // 2):
                            # half-major psum: each matmul writes one aligned
                            # 2KB bank (matmul outs must not cross banks)
                            pf = psF.tile([128, 2, NG, 128], F32, tag="pf")
                            for half in range(2):
                                blk = 2 * bp + half
                                mv = utg[:, blk::NB, :]
                                nc.tensor.matmul(pf[:, half, :, :], w0_t[:, :],
                                                 mv, start=True,
                                                 stop=(blk == 0))
                                if blk > 0:
                                    mv1 = utg[:, blk - 1::NB, :]
                                    nc.tensor.matmul(pf[:, half, :, :],
                                                     w1_t[:, :], mv1,
                                                     start=False, stop=True)
                            nc.scalar.copy(
                                vtg[:, :, 2 * bp:2 * bp + 2, :],
                                pf[:, :, :, :].rearrange("p h j e -> p j h e"))
                        # ---- transpose back: vtg [128t, (j,b,e)] -> vg
                        # C = j*NB + b ; vg[:, j*NB:(j+1)*NB, :] = v(n=g*NG+j)
                        vg = fir.tile([128, NG * NB, 128], BF16, tag="vg",
                                      bufs=2)
                        xpose(vg[:, :, :], vtg[:, :, :, :])
                        # ---- h-scans + y accumulation ----
                        for j in range(NG):
                            n = g * NG + j
                            cm_rep = rep.tile([128, L], BF16, tag="cm")
                            nc.gpsimd.dma_start(
                                out=cm_rep[:, :],
                                in_=bass.AP(tensor=dbc_ap.tensor,
                                            offset=(DT_RANK + N_ST + n) * L,
                                            ap=[[0, 128], [1, L]]))
                            a_t = sc.tile([128, L], BF16, tag="a")
                            nc.scalar.activation(a_t[:, :], ebl(delta_t, m),
                                                 AF.Exp,
                                                 scale=acols_t[:, m, n:n + 1])
                            h_t = sc.tile([128, L], BF16, tag="h")
                            nc.vector.tensor_tensor_scan(
                                h_t[:, :], a_t[:, :],
                                vg[:, j * NB:(j + 1) * NB, :].opt(),
                                0.0, OP.mult, OP.add)
                            yterm = sc.tile([128, L], BF16, tag="yt")
                            nc.vector.tensor_tensor(out=yterm[:, :],
                                                    in0=h_t[:, :],
                                                    in1=cm_rep[:, :],
                                                    op=OP.mult)
                            for t in range(NT):
                                nc.tensor.matmul(
                                    y_ps[:, t * 512:(t + 1) * 512],
                                    ident_t[:, :],
                                    yterm[:, t * 512:(t + 1) * 512],
                                    start=(n == 0), stop=(n == N_ST - 1))
                    # ---- y + D*xc, gate (per m; frees y psum for next m) ----
                    g_t = res.tile([128, NE, L], BF16, tag="g")
                    for t in range(NT):
                        tsl = slice(t * 512, (t + 1) * 512)
                        y_bf = sc.tile([128, 512], BF16, tag="ybf", bufs=3)
                        nc.scalar.copy(y_bf[:, :], y_ps[:, tsl])
                        yd = sc.tile([128, 512], BF16, tag="yd", bufs=3)
                        nc.vector.scalar_tensor_tensor(
                            yd[:, :], ebl(xc_t, m)[:, tsl],
                            dcol_t[:, m, :],
                            y_bf[:, :], OP.mult, OP.add)
                        nc.vector.tensor_tensor(out=ebl(g_t, m)[:, tsl],
                                                in0=yd[:, :],
                                                in1=ebl(zs_t, m)[:, tsl],
                                                op=OP.mult)

            # =================== out_proj ===================
            with (
                tc.tile_pool(name="oc", bufs=4) as oc,
                tc.tile_pool(name="psC", bufs=4, space="PSUM") as psC,
            ):
                for t in range(NT):
                    for mo in range(8):
                        ps = psC.tile([128, 512], F32, tag="pC")
                        for m in range(NE):
                            nc.tensor.matmul(
                                ps[:],
                                wout_t[:, m, mo * 128:(mo + 1) * 128],
                                ebl(g_t, m)[:, t * 512:(t + 1) * 512],
                                start=(m == 0), stop=(m == NE - 1))
                        ot = oc.tile([128, 512], F32, tag="ot")
                        nc.scalar.copy(ot[:, :], ps[:])
                        nc.gpsimd.dma_start(
                            out=out_pT[mo * 128:(mo + 1) * 128,
                                       t * 512:(t + 1) * 512],
                            in_=ot[:, :])

    _split_ctrl_waits(nc)
    return nc


def _get_programs():
    if "a" not in _CACHE:
        _CACHE["a"] = _build_a()
        _CACHE["b"] = _build_b()
    return _CACHE["a"], _CACHE["b"]


def _in_maps_a(x, W_in, conv_w, conv_b, W_x):
    x = np.asarray(x, np.float32)
    xT = np.ascontiguousarray(x[0].T)
    W_in = np.asarray(W_in, np.float32)
    maps = []
    for j in range(N_CORES):
        sl = slice(j * E, (j + 1) * E)
        maps.append({
            "xT": xT,
            "wxcT": np.ascontiguousarray(W_in[sl, :].T),
            "convw": np.ascontiguousarray(np.asarray(conv_w, np.float32)[sl]),
            "convb": np.ascontiguousarray(np.asarray(conv_b, np.float32)[sl])[:, None],
            "wxT": np.ascontiguousarray(np.asarray(W_x, np.float32)[:, sl].T),
        })
    return maps


def _fir_mats():
    """Banded Toeplitz taps for v = EMA_beta(u) truncated at 16 taps.
    W0[t', t] = beta^(t-t') for 0 <= t-t' < 16 (within time block);
    W1[t', t] = beta^(t+128-t') for 1 <= t+128-t' < 16 (carry from prev block).
    """
    J = 16
    w0 = np.zeros((128, 128), np.float32)
    w1 = np.zeros((128, 128), np.float32)
    for tp in range(128):
        for t in range(128):
            d = t - tp
            if 0 <= d < J:
                w0[tp, t] = BETA ** d
            d2 = t + 128 - tp
            if 1 <= d2 < J:
                w1[tp, t] = BETA ** d2
    return w0.astype(ml_dtypes.bfloat16), w1.astype(ml_dtypes.bfloat16)


def _in_maps_b(res_a, x, W_in, W_dt, b_dt, A_log, D, W_out):
    x = np.asarray(x, np.float32)
    xT = np.ascontiguousarray(x[0].T)
    W_in = np.asarray(W_in, np.float32)
    A = -np.exp(np.asarray(A_log, np.float32))
    ident = np.eye(128, dtype=ml_dtypes.bfloat16)
    w0, w1 = _fir_mats()
    dbc = np.zeros((DBC, L), np.float32)
    for j in range(N_CORES):
        dbc += np.asarray(res_a[j]["dbcp_o"], np.float32)
    dbc = dbc.astype(ml_dtypes.bfloat16)
    maps = []
    for j in range(N_CORES):
        sl = slice(j * E, (j + 1) * E)
        maps.append({
            "xc_i": res_a[j]["xc_o"],
            "xT": xT,
            "wzT": np.ascontiguousarray(W_in[ED + j * E:ED + (j + 1) * E, :].T),
            "dbc_i": dbc,
            "wdtT": np.ascontiguousarray(
                np.asarray(W_dt, np.float32)[sl, :].T).astype(ml_dtypes.bfloat16),
            "bdt": np.ascontiguousarray(np.asarray(b_dt, np.float32)[sl])[:, None],
            "acols": np.ascontiguousarray(A[sl, :]),
            "dcol": np.ascontiguousarray(np.asarray(D, np.float32)[sl])[:, None],
            "woutT": np.ascontiguousarray(
                np.asarray(W_out, np.float32)[:, sl].T).astype(ml_dtypes.bfloat16),
            "ident": ident,
            "w0fir": w0,
            "w1fir": w1,
        })
    return maps


def kernel(x, W_in, conv_w, conv_b, W_x, W_dt, b_dt, A_log, D, W_out):
    from concourse.bass_utils import run_bass_kernel_spmd

    nc_a, nc_b = _get_programs()
    res_a = run_bass_kernel_spmd(nc_a, _in_maps_a(x, W_in, conv_w, conv_b, W_x),
                                 list(range(N_CORES))).results
    res_b = run_bass_kernel_spmd(nc_b,
                                 _in_maps_b(res_a, x, W_in, W_dt, b_dt, A_log, D, W_out),
                                 list(range(N_CORES))).results
    out_T = np.zeros((D_MODEL, L), np.float64)
    for j in range(N_CORES):
        out_T += res_b[j]["out_pT"]
    return out_T.T[None, :, :].astype(np.float32)



# revision 18
# speedup vs baseline: 1.0013x; 1.0013x over previous
"""Trainium2 Bass kernel for MambaMomentum (B=1, L=2048, D=1024, ED=2048, N=16).

Tensor-parallel over d_inner (ED) across 8 NeuronCores; each core owns 256
channels end-to-end. The one cross-core dependency (dBC = xc @ W_x.T, a
full-ED contraction) is handled by splitting the kernel into two launches
with a host-side 8-way sum of the small (96 x 2048) partials between them —
the on-device AllReduce costs ~80us of latency-floor, the host reduce is
free.

Launch A: in_proj (f32r matmuls), depthwise causal conv, SiLU, x_proj
partials. Launch B: dt_proj/softplus, the (ED x N) selective scan with
momentum (DVE TensorTensorScan in bf16, channels on partitions, time on the
free dim), state reduction over N via PE identity-matmul accumulation in
PSUM, gating, out_proj partials (summed on host).
"""

import sys

if "/opt/trn_rl_repo" not in sys.path:
    sys.path.insert(0, "/opt/trn_rl_repo")

import numpy as np
import ml_dtypes

import concourse.bass as bass
import concourse.mybir as mybir
from concourse.tile import TileContext

N_CORES = 8
D_MODEL = 1024
ED = 2048
N_ST = 16
DT_RANK = 64
K_CONV = 4
BETA = 0.6
ALPHA = 1.0
L = 2048
E = ED // N_CORES  # 256
NE = E // 128      # 2
NT = L // 512      # 4
DBC = DT_RANK + 2 * N_ST  # 96
BF16 = mybir.dt.bfloat16
F32 = mybir.dt.float32
F32R = mybir.dt.float32r
AF = mybir.ActivationFunctionType
OP = mybir.AluOpType

_CACHE = {}


def _split_ctrl_waits(nc, max_waits=1):
    """walrus CoreV3 codegen rejects >1 sem-wait on several encodings; move
    excess waits onto single-wait NoOps inserted just before."""
    for fn in nc.m.functions:
        for bb in fn.blocks:
            new_insts = []
            for inst in bb.instructions:
                si = inst.sync_info
                if si is not None and si.on_wait and len(si.on_wait) > max_waits:
                    waits = list(si.on_wait)
                    si.on_wait = waits[:max_waits]
                    extra = waits[max_waits:]
                    for i in range(0, len(extra), max_waits):
                        new_insts.append(mybir.InstNoOp(
                            name=f"{inst.name}_ws{i}",
                            engine=inst.engine,
                            ins=[], outs=[],
                            sync_info=mybir.SyncInfo(
                                on_wait=extra[i:i + max_waits], on_update=[]),
                        ))
                new_insts.append(inst)
            bb.instructions[:] = new_insts


def _build_a():
    nc = bass.Bass("TRN2", target_bir_lowering=False, debug=False,
                   num_devices=N_CORES)
    xT = nc.dram_tensor("xT", [D_MODEL, L], F32R, kind="ExternalInput")
    wxcT = nc.dram_tensor("wxcT", [D_MODEL, E], F32R, kind="ExternalInput")
    convw = nc.dram_tensor("convw", [E, K_CONV], F32, kind="ExternalInput")
    convb = nc.dram_tensor("convb", [E, 1], F32, kind="ExternalInput")
    wxT = nc.dram_tensor("wxT", [E, DBC], F32R, kind="ExternalInput")
    xc_o = nc.dram_tensor("xc_o", [E, L], F32R, kind="ExternalOutput")
    dbcp_o = nc.dram_tensor("dbcp_o", [DBC, L], BF16, kind="ExternalOutput")

    with TileContext(nc) as tc:
        with (
            tc.tile_pool(name="prm", bufs=1) as prm,
            tc.tile_pool(name="xin", bufs=1) as xin,
            tc.tile_pool(name="wts", bufs=1) as wts,
            tc.tile_pool(name="stg", bufs=2) as stg,
            tc.tile_pool(name="stg1", bufs=1) as stg1,
            tc.tile_pool(name="psA", bufs=1, space="PSUM") as psA,
        ):
            w_in_t = wts.tile([128, 8, E], F32R, tag="w_in")
            x_t = xin.tile([128, 8, L], F32R, tag="x")
            for k in range(8):
                ksl = slice(k * 128, (k + 1) * 128)
                nc.sync.dma_start(out=w_in_t[:, k, :], in_=wxcT[ksl, :])
                nc.sync.dma_start(out=x_t[:, k, :], in_=xT[ksl, :])
            convw_t = prm.tile([128, NE, K_CONV], F32, tag="convw")
            convb_t = prm.tile([128, NE, 1], F32, tag="convb")
            wx_t = prm.tile([128, NE, DBC], F32R, tag="wx")
            for m in range(NE):
                sl = slice(m * 128, (m + 1) * 128)
                nc.gpsimd.dma_start(out=convw_t[:, m, :], in_=convw[sl, :])
                nc.gpsimd.dma_start(out=convb_t[:, m, :], in_=convb[sl, :])
                nc.gpsimd.dma_start(out=wx_t[:, m, :], in_=wxT[sl, :])

            # PE warm-up: ~4us of junk matmuls so in_proj runs at 2.4 GHz
            wu_ps = psA.tile([128, 512], F32, tag="pA00", name="warm_ps")
            for _w in range(20):
                nc.tensor.matmul(wu_ps[:], w_in_t[:, 0, 0:128],
                                 x_t[:, 0, 0:512], start=True, stop=True)

            xc_t = [None] * NE
            for m in range(NE):
                psx = [psA.tile([128, 512], F32, tag=f"pA{m}{t}",
                                name=f"psx{m}{t}") for t in range(NT)]
                for k in range(8):
                    for t in range(NT):
                        nc.tensor.matmul(psx[t][:],
                                         w_in_t[:, k, m * 128:(m + 1) * 128],
                                         x_t[:, k, t * 512:(t + 1) * 512],
                                         start=(k == 0), stop=(k == 7))
                raw = stg.tile([128, L], F32, tag="xcraw")
                for t in range(NT):
                    nc.scalar.copy(raw[:, t * 512:(t + 1) * 512], psx[t][:])
                acc = stg1.tile([128, L], F32, tag="convacc")
                cw = convw_t[:, m, :]
                nc.vector.tensor_scalar_mul(acc[:, :], raw[:, :], cw[:, 3:4])
                for kk in range(1, K_CONV):
                    nc.vector.scalar_tensor_tensor(
                        acc[:, kk:], raw[:, :L - kk], cw[:, 3 - kk:4 - kk],
                        acc[:, kk:], OP.mult, OP.add)
                xc_t[m] = stg1.tile([128, L], F32R, tag=f"xc{m}",
                                    name=f"xc_t{m}")
                nc.scalar.activation(xc_t[m][:, :], acc[:, :], AF.Silu,
                                     bias=convb_t[:, m, :], scale=1.0)
                nc.sync.dma_start(out=xc_o[m * 128:(m + 1) * 128, :],
                                  in_=xc_t[m][:, :])

            # x_proj partial
            for t in range(NT):
                ps = psA.tile([128, 512], F32, tag=f"pA0{t}", name=f"psb{t}")
                for m in range(NE):
                    nc.tensor.matmul(ps[0:DBC, :], wx_t[:, m, :],
                                     xc_t[m][:, t * 512:(t + 1) * 512],
                                     start=(m == 0), stop=(m == NE - 1))
                dst = stg.tile([DBC, 512], BF16, tag="dbcp")
                nc.scalar.copy(dst[:, :], ps[0:DBC, :])
                nc.sync.dma_start(out=dbcp_o[:, t * 512:(t + 1) * 512],
                                  in_=dst[:, :])

    _split_ctrl_waits(nc)
    return nc


def _build_b():
    from concourse.bass import _add_dep_helper

    nc = bass.Bass("TRN2", target_bir_lowering=False, debug=False,
                   num_devices=N_CORES)
    xc_i = nc.dram_tensor("xc_i", [E, L], F32R, kind="ExternalInput")
    xT = nc.dram_tensor("xT", [D_MODEL, L], F32R, kind="ExternalInput")
    wzT = nc.dram_tensor("wzT", [D_MODEL, E], F32R, kind="ExternalInput")
    dbc_i = nc.dram_tensor("dbc_i", [DBC, L], BF16, kind="ExternalInput")
    wdtT = nc.dram_tensor("wdtT", [DT_RANK, E], BF16, kind="ExternalInput")
    bdt = nc.dram_tensor("bdt", [E, 1], F32, kind="ExternalInput")
    acols = nc.dram_tensor("acols", [E, N_ST], F32, kind="ExternalInput")
    dcol = nc.dram_tensor("dcol", [E, 1], F32, kind="ExternalInput")
    woutT = nc.dram_tensor("woutT", [E, D_MODEL], BF16, kind="ExternalInput")
    ident = nc.dram_tensor("ident", [128, 128], BF16, kind="ExternalInput")
    w0fir = nc.dram_tensor("w0fir", [128, 128], BF16, kind="ExternalInput")
    w1fir = nc.dram_tensor("w1fir", [128, 128], BF16, kind="ExternalInput")
    out_pT = nc.dram_tensor("out_pT", [D_MODEL, L], F32, kind="ExternalOutput")
    dbc_ap = dbc_i.ap()
    NB = L // 128  # 16 time blocks of 128
    NG = 4         # n-group size for batched transpose / FIR

    def ebl(t3, m):
        return t3[:, m, :]

    # XBAR transposes corrupt each other when concurrent; chain-serialize all.
    tchain = []

    def xpose(out, in_):
        t = nc.sync.dma_start_transpose(out=out, in_=in_)
        if tchain:
            _add_dep_helper(t.ins, tchain[-1].ins, sync=True,
                            reason="xbar serialize")
        tchain.append(t)
        return t

    with TileContext(nc) as tc:
        with (
            tc.tile_pool(name="res", bufs=1) as res,
            tc.tile_pool(name="prm", bufs=1) as prm,
        ):
            xc_t = res.tile([128, NE, L], BF16, tag="xc")
            zs_t = res.tile([128, NE, L], BF16, tag="zs")
            delta_t = res.tile([128, NE, L], F32, tag="delta")
            wu_t = res.tile([128, NE, L], BF16, tag="wu")
            wout_t = res.tile([128, NE, D_MODEL], BF16, tag="wout")

            bdt_t = prm.tile([128, NE, 1], F32, tag="bdt")
            acols_t = prm.tile([128, NE, N_ST], F32, tag="acols")
            dcol_t = prm.tile([128, NE, 1], F32, tag="dcol")
            wdt_t = prm.tile([DT_RANK, E], BF16, tag="wdt")
            ident_t = prm.tile([128, 128], BF16, tag="ident")
            w0_t = prm.tile([128, 128], BF16, tag="w0")
            w1_t = prm.tile([128, 128], BF16, tag="w1")
            dbcd_t = prm.tile([DT_RANK, L], BF16, tag="dbcd")

            # order matters: the delta-chain inputs first
            nc.gpsimd.dma_start(out=dbcd_t[:, :], in_=dbc_i[0:DT_RANK, :])
            nc.gpsimd.dma_start(out=wdt_t[:, :], in_=wdtT[:, :])
            nc.gpsimd.dma_start(out=ident_t[:, :], in_=ident[:, :])
            nc.gpsimd.dma_start(out=w0_t[:, :], in_=w0fir[:, :])
            nc.gpsimd.dma_start(out=w1_t[:, :], in_=w1fir[:, :])
            for m in range(NE):
                sl = slice(m * 128, (m + 1) * 128)
                nc.gpsimd.dma_start(out=bdt_t[:, m, :], in_=bdt[sl, :])
                nc.gpsimd.dma_start(out=acols_t[:, m, :], in_=acols[sl, :])
                nc.gpsimd.dma_start(out=dcol_t[:, m, :], in_=dcol[sl, :])
            for m in range(NE):
                sl = slice(m * 128, (m + 1) * 128)
                nc.scalar.dma_start(out=wout_t[:, m, :], in_=woutT[sl, :])

            with (
                tc.tile_pool(name="stg2", bufs=2) as stg2,
                tc.tile_pool(name="xcl", bufs=1) as xcl,
                tc.tile_pool(name="psD", bufs=4, space="PSUM") as psD,
            ):
                xcf = xcl.tile([128, NE, L], F32R, tag="xcf")
                for m in range(NE):
                    sl = slice(m * 128, (m + 1) * 128)
                    nc.scalar.dma_start(out=ebl(xcf, m), in_=xc_i[sl, :])
                    nc.scalar.copy(ebl(xc_t, m), ebl(xcf, m).bitcast(F32))
                warmact = stg2.tile([128, 1], F32, tag="warmact")
                nc.scalar.activation(warmact[:, :], bdt_t[:, 0, :], AF.Exp)
                for m in range(NE):
                    dd = ebl(delta_t, m)
                    for t in range(NT):
                        ps = psD.tile([128, 512], F32, tag="pD")
                        nc.tensor.matmul(ps[:], wdt_t[:, m * 128:(m + 1) * 128],
                                         dbcd_t[:, t * 512:(t + 1) * 512],
                                         start=True, stop=True)
                        # softplus(x+b) = Ln(1+Exp(x+b)); x+b in [-9.3,-2.2]
                        nc.scalar.activation(dd[:, t * 512:(t + 1) * 512], ps[:],
                                             AF.Exp, bias=bdt_t[:, m, :], scale=1.0)
                    nc.vector.tensor_scalar_add(dd, dd, 1.0)
                    nc.scalar.activation(dd, dd, AF.Ln)
                    nc.vector.tensor_tensor(out=ebl(wu_t, m), in0=dd,
                                            in1=ebl(xcf, m).bitcast(F32),
                                            op=OP.mult)


            # =================== momentum-FIR + scan ===================
            # v = EMA_beta(u) == 16-tap FIR (err beta^16 ~ 2.8e-4) computed on
            # PE in time-major. u_T is built directly on DVE from wu_T
            # (2 xbar transposes per core) with bm columns as per-partition
            # tensor_scalar multipliers, so only the v-side transposes remain
            # on the serialized XBAR. z-proj streams through the y psum banks.
            with (
                tc.tile_pool(name="pbz", bufs=1) as pbz,
                tc.tile_pool(name="repz", bufs=2) as repz,
                tc.tile_pool(name="fir", bufs=1) as fir,
                tc.tile_pool(name="rep", bufs=2) as rep,
                tc.tile_pool(name="sc", bufs=2) as sc,
                tc.tile_pool(name="psF", bufs=2, space="PSUM") as psF,
                tc.tile_pool(name="psY", bufs=1, space="PSUM") as psY,
            ):
                wz_t = pbz.tile([128, 8, E], F32R, tag="wz")
                for k in range(8):
                    nc.gpsimd.dma_start(out=wz_t[:, k, :],
                                        in_=wzT[k * 128:(k + 1) * 128, :])
                # wu_T: [128 t, m, b, 128 e]; bm columns: [128 t, n, b]
                wuT = pbz.tile([128, NE, NB, 128], BF16, tag="wuT")
                for m in range(NE):
                    xpose(wuT[:, m, :, :], ebl(wu_t, m))
                # bm rows -> per-(t,blk) columns via one tiny XBAR transpose
                bmcolb = pbz.tile([128, NB, N_ST], BF16, tag="bmcolb")
                xpose(bmcolb[:, :, :], dbc_i[DT_RANK:DT_RANK + N_ST, :])
                bmcol = pbz.tile([128, NB, N_ST], F32, tag="bmcol")
                nc.scalar.copy(bmcol[:, :, :], bmcolb[:, :, :])
                for m in range(NE):
                    y_ps = psY.tile([128, L], F32, tag="y", name=f"y_ps{m}")
                    # ---- z-proj(m) through the y psum banks, silu to zs ----
                    for k in range(8):
                        xbk = repz.tile([128, L], F32R, tag="xbk")
                        nc.gpsimd.dma_start(
                            out=xbk[:, :],
                            in_=xT[k * 128:(k + 1) * 128, :])
                        for t in range(NT):
                            nc.tensor.matmul(
                                y_ps[:, t * 512:(t + 1) * 512],
                                wz_t[:, k, m * 128:(m + 1) * 128],
                                xbk[:, t * 512:(t + 1) * 512],
                                start=(k == 0), stop=(k == 7))
                    for t in range(NT):
                        nc.scalar.activation(
                            ebl(zs_t, m)[:, t * 512:(t + 1) * 512],
                            y_ps[:, t * 512:(t + 1) * 512], AF.Silu)
                    for g in range(N_ST // NG):
                        # ---- u_T built directly: tensor_scalar per (n, blk)
                        utg = fir.tile([128, NG, NB, 128], BF16, tag="utg",
                                       bufs=2)
                        for j in range(NG):
                            n = g * NG + j
                            for b in range(NB):
                                nc.vector.tensor_scalar_mul(
                                    utg[:, j, b, :], wuT[:, m, b, :],
                                    bmcol[:, b, n:n + 1])
                        # ---- PE FIR into psum, 2 time blocks per psum tile ----
                        # psum + vtg free order: [j][b][e] so each n's v is
                        # contiguous after the reverse transpose
                        vtg = fir.tile([128, NG, NB, 128], BF16, tag="vtg")
                        for bp in range(NB // 2):
                            # half-major psum: each matmul writes one aligned
                            # 2KB bank (matmul outs must not cross banks)
                            pf = psF.tile([128, 2, NG, 128], F32, tag="pf")
                            for half in range(2):
                                blk = 2 * bp + half
                                mv = utg[:, :, blk, :]
                                nc.tensor.matmul(pf[:, half, :, :], w0_t[:, :],
                                                 mv, start=True,
                                                 stop=(blk == 0))
                                if blk > 0:
                                    mv1 = utg[:, :, blk - 1, :]
                                    nc.tensor.matmul(pf[:, half, :, :],
                                                     w1_t[:, :], mv1,
                                                     start=False, stop=True)
                            nc.scalar.copy(
                                vtg[:, :, 2 * bp:2 * bp + 2, :],
                                pf[:, :, :, :].rearrange("p h j e -> p j h e"))
                        # ---- transpose back: vtg [128t, (j,b,e)] -> vg
                        # C = j*NB + b ; vg[:, j*NB:(j+1)*NB, :] = v(n=g*NG+j)
                        vg = fir.tile([128, NG * NB, 128], BF16, tag="vg",
                                      bufs=2)
                        xpose(vg[:, :, :], vtg[:, :, :, :])
                        # ---- h-scans + y accumulation ----
                        for j in range(NG):
                            n = g * NG + j
                            cm_rep = rep.tile([128, L], BF16, tag="cm")
                            nc.gpsimd.dma_start(
                                out=cm_rep[:, :],
                                in_=bass.AP(tensor=dbc_ap.tensor,
                                            offset=(DT_RANK + N_ST + n) * L,
                                            ap=[[0, 128], [1, L]]))
                            a_t = sc.tile([128, L], BF16, tag="a")
                            nc.scalar.activation(a_t[:, :], ebl(delta_t, m),
                                                 AF.Exp,
                                                 scale=acols_t[:, m, n:n + 1])
                            h_t = sc.tile([128, L], BF16, tag="h")
                            nc.vector.tensor_tensor_scan(
                                h_t[:, :], a_t[:, :],
                                vg[:, j * NB:(j + 1) * NB, :].opt(),
                                0.0, OP.mult, OP.add)
                            yterm = sc.tile([128, L], BF16, tag="yt")
                            nc.vector.tensor_tensor(out=yterm[:, :],
                                                    in0=h_t[:, :],
                                                    in1=cm_rep[:, :],
                                                    op=OP.mult)
                            for t in range(NT):
                                nc.tensor.matmul(
                                    y_ps[:, t * 512:(t + 1) * 512],
                                    ident_t[:, :],
                                    yterm[:, t * 512:(t + 1) * 512],
                                    start=(n == 0), stop=(n == N_ST - 1))
                    # ---- y + D*xc, gate (per m; frees y psum for next m) ----
                    g_t = res.tile([128, NE, L], BF16, tag="g")
                    for t in range(NT):
                        tsl = slice(t * 512, (t + 1) * 512)
                        y_bf = sc.tile([128, 512], BF16, tag="ybf", bufs=2)
                        nc.scalar.copy(y_bf[:, :], y_ps[:, tsl])
                        yd = sc.tile([128, 512], BF16, tag="yd", bufs=2)
                        nc.vector.scalar_tensor_tensor(
                            yd[:, :], ebl(xc_t, m)[:, tsl],
                            dcol_t[:, m, :],
                            y_bf[:, :], OP.mult, OP.add)
                        nc.vector.tensor_tensor(out=ebl(g_t, m)[:, tsl],
                                                in0=yd[:, :],
                                                in1=ebl(zs_t, m)[:, tsl],
                                                op=OP.mult)

            # =================== out_proj ===================
            with (
                tc.tile_pool(name="oc", bufs=4) as oc,
                tc.tile_pool(name="psC", bufs=4, space="PSUM") as psC,
            ):
                for t in range(NT):
                    for mo in range(8):
                        ps = psC.tile([128, 512], F32, tag="pC")
                        for m in range(NE):
                            nc.tensor.matmul(
                                ps[:],
                                wout_t[:, m, mo * 128:(mo + 1) * 128],
                                ebl(g_t, m)[:, t * 512:(t + 1) * 512],
                                start=(m == 0), stop=(m == NE - 1))
                        ot = oc.tile([128, 512], F32, tag="ot")
                        nc.scalar.copy(ot[:, :], ps[:])
                        nc.gpsimd.dma_start(
                            out=out_pT[mo * 128:(mo + 1) * 128,
                                       t * 512:(t + 1) * 512],
                            in_=ot[:, :])

    _split_ctrl_waits(nc)
    return nc


def _get_programs():
    if "a" not in _CACHE:
        _CACHE["a"] = _build_a()
        _CACHE["b"] = _build_b()
    return _CACHE["a"], _CACHE["b"]


def _in_maps_a(x, W_in, conv_w, conv_b, W_x):
    x = np.asarray(x, np.float32)
    xT = np.ascontiguousarray(x[0].T)
    W_in = np.asarray(W_in, np.float32)
    maps = []
    for j in range(N_CORES):
        sl = slice(j * E, (j + 1) * E)
        maps.append({
            "xT": xT,
            "wxcT": np.ascontiguousarray(W_in[sl, :].T),
            "convw": np.ascontiguousarray(np.asarray(conv_w, np.float32)[sl]),
            "convb": np.ascontiguousarray(np.asarray(conv_b, np.float32)[sl])[:, None],
            "wxT": np.ascontiguousarray(np.asarray(W_x, np.float32)[:, sl].T),
        })
    return maps


def _fir_mats():
    """Banded Toeplitz taps for v = EMA_beta(u) truncated at 16 taps.
    W0[t', t] = beta^(t-t') for 0 <= t-t' < 16 (within time block);
    W1[t', t] = beta^(t+128-t') for 1 <= t+128-t' < 16 (carry from prev block).
    """
    J = 16
    w0 = np.zeros((128, 128), np.float32)
    w1 = np.zeros((128, 128), np.float32)
    for tp in range(128):
        for t in range(128):
            d = t - tp
            if 0 <= d < J:
                w0[tp, t] = BETA ** d
            d2 = t + 128 - tp
            if 1 <= d2 < J:
                w1[tp, t] = BETA ** d2
    return w0.astype(ml_dtypes.bfloat16), w1.astype(ml_dtypes.bfloat16)


def _in_maps_b(res_a, x, W_in, W_dt, b_dt, A_log, D, W_out):
    x = np.asarray(x, np.float32)
    xT = np.ascontiguousarray(x[0].T)
    W_in = np.asarray(W_in, np.float32)
    A = -np.exp(np.asarray(A_log, np.float32))
    ident = np.eye(128, dtype=ml_dtypes.bfloat16)
    w0, w1 = _fir_mats()
    dbc = np.zeros((DBC, L), np.float32)
    for j in range(N_CORES):
        dbc += np.asarray(res_a[j]["dbcp_o"], np.float32)
    dbc = dbc.astype(ml_dtypes.bfloat16)
    maps = []
    for j in range(N_CORES):
        sl = slice(j * E, (j + 1) * E)
        maps.append({
            "xc_i": res_a[j]["xc_o"],
            "xT": xT,
            "wzT": np.ascontiguousarray(W_in[ED + j * E:ED + (j + 1) * E, :].T),
            "dbc_i": dbc,
            "wdtT": np.ascontiguousarray(
                np.asarray(W_dt, np.float32)[sl, :].T).astype(ml_dtypes.bfloat16),
            "bdt": np.ascontiguousarray(np.asarray(b_dt, np.float32)[sl])[:, None],
            "acols": np.ascontiguousarray(A[sl, :]),
            "dcol": np.ascontiguousarray(np.asarray(D, np.float32)[sl])[:, None],
            "woutT": np.ascontiguousarray(
                np.asarray(W_out, np.float32)[:, sl].T).astype(ml_dtypes.bfloat16),
            "ident": ident,
            "w0fir": w0,
            "w1fir": w1,
        })
    return maps


def kernel(x, W_in, conv_w, conv_b, W_x, W_dt, b_dt, A_log, D, W_out):
    from concourse.bass_utils import run_bass_kernel_spmd

    nc_a, nc_b = _get_programs()
    res_a = run_bass_kernel_spmd(nc_a, _in_maps_a(x, W_in, conv_w, conv_b, W_x),
                                 list(range(N_CORES))).results
    res_b = run_bass_kernel_spmd(nc_b,
                                 _in_maps_b(res_a, x, W_in, W_dt, b_dt, A_log, D, W_out),
                                 list(range(N_CORES))).results
    out_T = np.zeros((D_MODEL, L), np.float64)
    for j in range(N_CORES):
        out_T += res_b[j]["out_pT"]
    return out_T.T[None, :, :].astype(np.float32)



# revision 20
# speedup vs baseline: 1.0100x; 1.0087x over previous
"""Trainium2 Bass kernel for MambaMomentum (B=1, L=2048, D=1024, ED=2048, N=16).

Tensor-parallel over d_inner (ED) across 8 NeuronCores; each core owns 256
channels end-to-end. The one cross-core dependency (dBC = xc @ W_x.T, a
full-ED contraction) is handled by splitting the kernel into two launches
with a host-side 8-way sum of the small (96 x 2048) partials between them —
the on-device AllReduce costs ~80us of latency-floor, the host reduce is
free.

Launch A: in_proj (both xc and z halves, f32r matmuls), depthwise causal
conv, SiLU on both paths, x_proj partials. Launch B: dt_proj/softplus, the
(ED x N) selective scan with momentum, gating, out_proj partials (summed on
host).

The momentum recurrence v = EMA_beta(u) is NOT a DVE scan: with constant
beta it is a 16-tap FIR (truncation error beta^16 ~ 2.8e-4), computed on
the otherwise-idle PE as banded-Toeplitz matmuls in time-major layout.
u (channel-major) is moved to time-major and v back via the hardware XBAR
transpose DMA. XBAR transposes corrupt each other when concurrent, so all
transposes are serialized on one dependency chain; the group pipeline is
software-pipelined (u-transpose of group k+1 is chained before v-transpose
of group k) so the XBAR, PE FIR, DVE h-scans and ACT copies all overlap.
DVE then only runs the 32 h-scans plus the u/y elementwise multiplies.
"""

import sys

if "/opt/trn_rl_repo" not in sys.path:
    sys.path.insert(0, "/opt/trn_rl_repo")

import numpy as np
import ml_dtypes

import concourse.bass as bass
import concourse.mybir as mybir
from concourse.tile import TileContext

N_CORES = 8
D_MODEL = 1024
ED = 2048
N_ST = 16
DT_RANK = 64
K_CONV = 4
BETA = 0.6
ALPHA = 1.0
L = 2048
E = ED // N_CORES  # 256
NE = E // 128      # 2
NT = L // 512      # 4
NB = L // 128      # 16
NG = 4             # n-group size for batched transpose / FIR
DBC = DT_RANK + 2 * N_ST  # 96
BF16 = mybir.dt.bfloat16
F32 = mybir.dt.float32
F32R = mybir.dt.float32r
AF = mybir.ActivationFunctionType
OP = mybir.AluOpType

_CACHE = {}


def _split_ctrl_waits(nc, max_waits=1):
    """walrus CoreV3 codegen rejects >1 sem-wait on several encodings; move
    excess waits onto single-wait NoOps inserted just before."""
    for fn in nc.m.functions:
        for bb in fn.blocks:
            new_insts = []
            for inst in bb.instructions:
                si = inst.sync_info
                if si is not None and si.on_wait and len(si.on_wait) > max_waits:
                    waits = list(si.on_wait)
                    si.on_wait = waits[:max_waits]
                    extra = waits[max_waits:]
                    for i in range(0, len(extra), max_waits):
                        new_insts.append(mybir.InstNoOp(
                            name=f"{inst.name}_ws{i}",
                            engine=inst.engine,
                            ins=[], outs=[],
                            sync_info=mybir.SyncInfo(
                                on_wait=extra[i:i + max_waits], on_update=[]),
                        ))
                new_insts.append(inst)
            bb.instructions[:] = new_insts


def _build_a():
    nc = bass.Bass("TRN2", target_bir_lowering=False, debug=False,
                   num_devices=N_CORES)
    xT = nc.dram_tensor("xT", [D_MODEL, L], F32R, kind="ExternalInput")
    wxcT = nc.dram_tensor("wxcT", [D_MODEL, E], F32R, kind="ExternalInput")
    wzT = nc.dram_tensor("wzT", [D_MODEL, E], F32R, kind="ExternalInput")
    convw = nc.dram_tensor("convw", [E, K_CONV], F32, kind="ExternalInput")
    convb = nc.dram_tensor("convb", [E, 1], F32, kind="ExternalInput")
    wxT = nc.dram_tensor("wxT", [E, DBC], F32R, kind="ExternalInput")
    xc_o = nc.dram_tensor("xc_o", [E, L], F32R, kind="ExternalOutput")
    zs_o = nc.dram_tensor("zs_o", [E, L], BF16, kind="ExternalOutput")
    dbcp_o = nc.dram_tensor("dbcp_o", [DBC, L], BF16, kind="ExternalOutput")

    with TileContext(nc) as tc:
        with (
            tc.tile_pool(name="prm", bufs=1) as prm,
            tc.tile_pool(name="xin", bufs=1) as xin,
            tc.tile_pool(name="wts", bufs=1) as wts,
            tc.tile_pool(name="stg", bufs=2) as stg,
            tc.tile_pool(name="stg1", bufs=1) as stg1,
            tc.tile_pool(name="psA", bufs=1, space="PSUM") as psA,
        ):
            w_in_t = wts.tile([128, 8, E], F32R, tag="w_in")
            wz_t = wts.tile([128, 8, E], F32R, tag="wz")
            x_t = xin.tile([128, 8, L], F32R, tag="x")
            for k in range(8):
                ksl = slice(k * 128, (k + 1) * 128)
                nc.sync.dma_start(out=w_in_t[:, k, :], in_=wxcT[ksl, :])
                nc.sync.dma_start(out=x_t[:, k, :], in_=xT[ksl, :])
                nc.scalar.dma_start(out=wz_t[:, k, :], in_=wzT[ksl, :])
            convw_t = prm.tile([128, NE, K_CONV], F32, tag="convw")
            convb_t = prm.tile([128, NE, 1], F32, tag="convb")
            wx_t = prm.tile([128, NE, DBC], F32R, tag="wx")
            for m in range(NE):
                sl = slice(m * 128, (m + 1) * 128)
                nc.gpsimd.dma_start(out=convw_t[:, m, :], in_=convw[sl, :])
                nc.gpsimd.dma_start(out=convb_t[:, m, :], in_=convb[sl, :])
                nc.gpsimd.dma_start(out=wx_t[:, m, :], in_=wxT[sl, :])

            # PE warm-up: ~4us of junk matmuls so in_proj runs at 2.4 GHz
            wu_ps = psA.tile([128, 512], F32, tag="pA00", name="warm_ps")
            for _w in range(20):
                nc.tensor.matmul(wu_ps[:], w_in_t[:, 0, 0:128],
                                 x_t[:, 0, 0:512], start=True, stop=True)

            xc_t = [None] * NE
            for m in range(NE):
                psx = [psA.tile([128, 512], F32, tag=f"pA{m}{t}",
                                name=f"psx{m}{t}") for t in range(NT)]
                for k in range(8):
                    for t in range(NT):
                        nc.tensor.matmul(psx[t][:],
                                         w_in_t[:, k, m * 128:(m + 1) * 128],
                                         x_t[:, k, t * 512:(t + 1) * 512],
                                         start=(k == 0), stop=(k == 7))
                raw = stg.tile([128, L], F32, tag="xcraw")
                for t in range(NT):
                    nc.scalar.copy(raw[:, t * 512:(t + 1) * 512], psx[t][:])
                acc = stg1.tile([128, L], F32, tag="convacc")
                cw = convw_t[:, m, :]
                nc.vector.tensor_scalar_mul(acc[:, :], raw[:, :], cw[:, 3:4])
                for kk in range(1, K_CONV):
                    nc.vector.scalar_tensor_tensor(
                        acc[:, kk:], raw[:, :L - kk], cw[:, 3 - kk:4 - kk],
                        acc[:, kk:], OP.mult, OP.add)
                xc_t[m] = stg1.tile([128, L], F32R, tag=f"xc{m}",
                                    name=f"xc_t{m}")
                nc.scalar.activation(xc_t[m][:, :], acc[:, :], AF.Silu,
                                     bias=convb_t[:, m, :], scale=1.0)
                nc.sync.dma_start(out=xc_o[m * 128:(m + 1) * 128, :],
                                  in_=xc_t[m][:, :])

            # x_proj partial
            for t in range(NT):
                ps = psA.tile([128, 512], F32, tag=f"pA0{t}", name=f"psb{t}")
                for m in range(NE):
                    nc.tensor.matmul(ps[0:DBC, :], wx_t[:, m, :],
                                     xc_t[m][:, t * 512:(t + 1) * 512],
                                     start=(m == 0), stop=(m == NE - 1))
                dst = stg.tile([DBC, 512], BF16, tag="dbcp")
                nc.scalar.copy(dst[:, :], ps[0:DBC, :])
                nc.sync.dma_start(out=dbcp_o[:, t * 512:(t + 1) * 512],
                                  in_=dst[:, :])

            # z half of in_proj + SiLU (reuses the pA1* psum tiles)
            for m in range(NE):
                for t in range(NT):
                    ps = psA.tile([128, 512], F32, tag=f"pA1{t}",
                                  name=f"psz{m}{t}")
                    for k in range(8):
                        nc.tensor.matmul(ps[:],
                                         wz_t[:, k, m * 128:(m + 1) * 128],
                                         x_t[:, k, t * 512:(t + 1) * 512],
                                         start=(k == 0), stop=(k == 7))
                    zst = stg.tile([128, 512], BF16, tag="zst")
                    nc.scalar.activation(zst[:, :], ps[:], AF.Silu)
                    nc.gpsimd.dma_start(
                        out=zs_o[m * 128:(m + 1) * 128,
                                 t * 512:(t + 1) * 512],
                        in_=zst[:, :])

    _split_ctrl_waits(nc)
    return nc


def _build_b():
    from concourse.bass import _add_dep_helper

    nc = bass.Bass("TRN2", target_bir_lowering=False, debug=False,
                   num_devices=N_CORES)
    xc_i = nc.dram_tensor("xc_i", [E, L], F32R, kind="ExternalInput")
    zs_i = nc.dram_tensor("zs_i", [E, L], BF16, kind="ExternalInput")
    dbc_i = nc.dram_tensor("dbc_i", [DBC, L], BF16, kind="ExternalInput")
    wdtT = nc.dram_tensor("wdtT", [DT_RANK, E], BF16, kind="ExternalInput")
    bdt = nc.dram_tensor("bdt", [E, 1], F32, kind="ExternalInput")
    acols = nc.dram_tensor("acols", [E, N_ST], F32, kind="ExternalInput")
    dcol = nc.dram_tensor("dcol", [E, 1], F32, kind="ExternalInput")
    woutT = nc.dram_tensor("woutT", [E, D_MODEL], BF16, kind="ExternalInput")
    ident = nc.dram_tensor("ident", [128, 128], BF16, kind="ExternalInput")
    w0fir = nc.dram_tensor("w0fir", [128, 128], BF16, kind="ExternalInput")
    w1fir = nc.dram_tensor("w1fir", [128, 128], BF16, kind="ExternalInput")
    out_pT = nc.dram_tensor("out_pT", [D_MODEL, L], F32, kind="ExternalOutput")
    dbc_ap = dbc_i.ap()

    def ebl(t3, m):
        return t3[:, m, :]

    # XBAR transposes corrupt each other when concurrent; chain-serialize all.
    tchain = []

    def xpose(out, in_):
        t = nc.sync.dma_start_transpose(out=out, in_=in_)
        if tchain:
            _add_dep_helper(t.ins, tchain[-1].ins, sync=True,
                            reason="xbar serialize")
        tchain.append(t)
        return t

    with TileContext(nc) as tc:
        with (
            tc.tile_pool(name="res", bufs=1) as res,
            tc.tile_pool(name="prm", bufs=1) as prm,
        ):
            xc_t = res.tile([128, NE, L], BF16, tag="xc")
            zs_t = res.tile([128, L], BF16, tag="zs")
            delta_t = res.tile([128, NE, L], F32, tag="delta")
            wu_t = res.tile([128, NE, L], BF16, tag="wu")
            wout_t = res.tile([128, NE, D_MODEL], BF16, tag="wout")

            bdt_t = prm.tile([128, NE, 1], F32, tag="bdt")
            acols_t = prm.tile([128, NE, N_ST], F32, tag="acols")
            dcol_t = prm.tile([128, NE, 1], F32, tag="dcol")
            wdt_t = prm.tile([DT_RANK, E], BF16, tag="wdt")
            ident_t = prm.tile([128, 128], BF16, tag="ident")
            w0_t = prm.tile([128, 128], BF16, tag="w0")
            w1_t = prm.tile([128, 128], BF16, tag="w1")

            nc.gpsimd.dma_start(out=wdt_t[:, :], in_=wdtT[:, :])
            nc.gpsimd.dma_start(out=ident_t[:, :], in_=ident[:, :])
            nc.gpsimd.dma_start(out=w0_t[:, :], in_=w0fir[:, :])
            nc.gpsimd.dma_start(out=w1_t[:, :], in_=w1fir[:, :])
            for m in range(NE):
                sl = slice(m * 128, (m + 1) * 128)
                nc.gpsimd.dma_start(out=bdt_t[:, m, :], in_=bdt[sl, :])
                nc.gpsimd.dma_start(out=acols_t[:, m, :], in_=acols[sl, :])
                nc.gpsimd.dma_start(out=dcol_t[:, m, :], in_=dcol[sl, :])
            for m in range(NE):
                sl = slice(m * 128, (m + 1) * 128)
                nc.scalar.dma_start(out=wout_t[:, m, :], in_=woutT[sl, :])

            # ---- dt_proj -> softplus -> delta ; wu = delta * xc ----
            with (
                tc.tile_pool(name="stg2", bufs=2) as stg2,
                tc.tile_pool(name="xcl", bufs=1) as xcl,
                tc.tile_pool(name="psD", bufs=4, space="PSUM") as psD,
            ):
                dbcd_t = xcl.tile([DT_RANK, L], BF16, tag="dbcd")
                nc.gpsimd.dma_start(out=dbcd_t[:, :], in_=dbc_i[0:DT_RANK, :])
                xcf = xcl.tile([128, NE, L], F32R, tag="xcf")
                for m in range(NE):
                    sl = slice(m * 128, (m + 1) * 128)
                    nc.scalar.dma_start(out=ebl(xcf, m), in_=xc_i[sl, :])
                    nc.scalar.copy(ebl(xc_t, m), ebl(xcf, m).bitcast(F32))
                warmact = stg2.tile([128, 1], F32, tag="warmact")
                nc.scalar.activation(warmact[:, :], bdt_t[:, 0, :], AF.Exp)
                for m in range(NE):
                    dd = ebl(delta_t, m)
                    for t in range(NT):
                        ps = psD.tile([128, 512], F32, tag="pD")
                        nc.tensor.matmul(ps[:], wdt_t[:, m * 128:(m + 1) * 128],
                                         dbcd_t[:, t * 512:(t + 1) * 512],
                                         start=True, stop=True)
                        # softplus(x+b) = Ln(1+Exp(x+b)); x+b in [-9.3,-2.2]
                        nc.scalar.activation(dd[:, t * 512:(t + 1) * 512], ps[:],
                                             AF.Exp, bias=bdt_t[:, m, :], scale=1.0)
                    nc.vector.tensor_scalar_add(dd, dd, 1.0)
                    nc.scalar.activation(dd, dd, AF.Ln)
                    nc.vector.tensor_tensor(out=ebl(wu_t, m), in0=dd,
                                            in1=ebl(xcf, m).bitcast(F32),
                                            op=OP.mult)

            # =================== momentum-FIR + h-scan ===================
            with (
                tc.tile_pool(name="fir", bufs=1) as fir,
                tc.tile_pool(name="rep", bufs=2) as rep,
                tc.tile_pool(name="sc", bufs=2) as sc,
                tc.tile_pool(name="psF", bufs=2, space="PSUM") as psF,
                tc.tile_pool(name="psY", bufs=1, space="PSUM") as psY,
            ):
                g_t = res.tile([128, NE, L], BF16, tag="g")
                nc.gpsimd.dma_start(out=zs_t[:, :], in_=zs_i[0:128, :])

                y_ps = {}

                def emit_head(m, g):
                    """u = wu*bm for the group, then batched u-transpose."""
                    ug = fir.tile([128, NG, L], BF16, tag="ug")
                    for j in range(NG):
                        n = g * NG + j
                        bm_rep = rep.tile([128, L], BF16, tag="bm")
                        nc.gpsimd.dma_start(
                            out=bm_rep[:, :],
                            in_=bass.AP(tensor=dbc_ap.tensor,
                                        offset=(DT_RANK + n) * L,
                                        ap=[[0, 128], [1, L]]))
                        nc.vector.tensor_tensor(out=ug[:, j, :],
                                                in0=ebl(wu_t, m),
                                                in1=bm_rep[:, :], op=OP.mult)
                    utg = fir.tile([128, NG * NB, 128], BF16, tag="utg",
                                   bufs=2)
                    xpose(utg[:, :, :], ug[:, :, :])
                    return utg

                def emit_tail(m, g, utg):
                    """FIR matmuls, psum->sbuf, v-transpose, h-scans, y acc."""
                    vtg = fir.tile([128, NG, NB, 128], BF16, tag="vtg")
                    for bp in range(NB // 2):
                        # half-major psum: each matmul writes one aligned
                        # 2KB bank (matmul outs must not cross banks)
                        pf = psF.tile([128, 2, NG, 128], F32, tag="pf")
                        for half in range(2):
                            blk = 2 * bp + half
                            nc.tensor.matmul(pf[:, half, :, :], w0_t[:, :],
                                             utg[:, blk::NB, :], start=True,
                                             stop=(blk == 0))
                            if blk > 0:
                                nc.tensor.matmul(pf[:, half, :, :], w1_t[:, :],
                                                 utg[:, blk - 1::NB, :],
                                                 start=False, stop=True)
                        nc.scalar.copy(
                            vtg[:, :, 2 * bp:2 * bp + 2, :],
                            pf[:, :, :, :].rearrange("p h j e -> p j h e"))
                    vg = fir.tile([128, NG * NB, 128], BF16, tag="vg", bufs=2)
                    xpose(vg[:, :, :], vtg[:, :, :, :])
                    for j in range(NG):
                        n = g * NG + j
                        cm_rep = rep.tile([128, L], BF16, tag="cm")
                        nc.gpsimd.dma_start(
                            out=cm_rep[:, :],
                            in_=bass.AP(tensor=dbc_ap.tensor,
                                        offset=(DT_RANK + N_ST + n) * L,
                                        ap=[[0, 128], [1, L]]))
                        a_t = sc.tile([128, L], BF16, tag="a")
                        nc.scalar.activation(a_t[:, :], ebl(delta_t, m),
                                             AF.Exp,
                                             scale=acols_t[:, m, n:n + 1])
                        h_t = sc.tile([128, L], BF16, tag="h")
                        nc.vector.tensor_tensor_scan(
                            h_t[:, :], a_t[:, :],
                            vg[:, j * NB:(j + 1) * NB, :].opt(),
                            0.0, OP.mult, OP.add)
                        yterm = sc.tile([128, L], BF16, tag="yt")
                        nc.vector.tensor_tensor(out=yterm[:, :], in0=h_t[:, :],
                                                in1=cm_rep[:, :], op=OP.mult)
                        for t in range(NT):
                            nc.tensor.matmul(
                                y_ps[m][:, t * 512:(t + 1) * 512],
                                ident_t[:, :],
                                yterm[:, t * 512:(t + 1) * 512],
                                start=(n == 0), stop=(n == N_ST - 1))

                def emit_gate(m):
                    """y + D*xc, gate with silu(z) (loaded from launch A)."""
                    for t in range(NT):
                        tsl = slice(t * 512, (t + 1) * 512)
                        y_bf = sc.tile([128, 512], BF16, tag="ybf", bufs=2)
                        nc.scalar.copy(y_bf[:, :], y_ps[m][:, tsl])
                        yd = sc.tile([128, 512], BF16, tag="yd", bufs=2)
                        nc.vector.scalar_tensor_tensor(
                            yd[:, :], ebl(xc_t, m)[:, tsl], dcol_t[:, m, :],
                            y_bf[:, :], OP.mult, OP.add)
                        nc.vector.tensor_tensor(out=ebl(g_t, m)[:, tsl],
                                                in0=yd[:, :],
                                                in1=zs_t[:, tsl], op=OP.mult)
                    if m + 1 < NE:
                        nc.gpsimd.dma_start(
                            out=zs_t[:, :],
                            in_=zs_i[(m + 1) * 128:(m + 2) * 128, :])

                groups = [(m, g) for m in range(NE) for g in range(N_ST // NG)]
                prev = None
                for (m, g) in groups:
                    if g == 0:
                        y_ps[m] = psY.tile([128, L], F32, tag="y",
                                           name=f"y_ps{m}")
                    utg = emit_head(m, g)
                    if prev is not None:
                        emit_tail(*prev)
                        if prev[1] == N_ST // NG - 1:
                            emit_gate(prev[0])
                    prev = (m, g, utg)
                emit_tail(*prev)
                emit_gate(prev[0])

            # =================== out_proj ===================
            with (
                tc.tile_pool(name="oc", bufs=4) as oc,
                tc.tile_pool(name="psC", bufs=4, space="PSUM") as psC,
            ):
                for t in range(NT):
                    for mo in range(8):
                        ps = psC.tile([128, 512], F32, tag="pC")
                        for m in range(NE):
                            nc.tensor.matmul(
                                ps[:],
                                wout_t[:, m, mo * 128:(mo + 1) * 128],
                                ebl(g_t, m)[:, t * 512:(t + 1) * 512],
                                start=(m == 0), stop=(m == NE - 1))
                        ot = oc.tile([128, 512], F32, tag="ot")
                        nc.scalar.copy(ot[:, :], ps[:])
                        nc.gpsimd.dma_start(
                            out=out_pT[mo * 128:(mo + 1) * 128,
                                       t * 512:(t + 1) * 512],
                            in_=ot[:, :])

    _split_ctrl_waits(nc)
    return nc


def _get_programs():
    if "a" not in _CACHE:
        _CACHE["a"] = _build_a()
        _CACHE["b"] = _build_b()
    return _CACHE["a"], _CACHE["b"]


def _in_maps_a(x, W_in, conv_w, conv_b, W_x):
    x = np.asarray(x, np.float32)
    xT = np.ascontiguousarray(x[0].T)
    W_in = np.asarray(W_in, np.float32)
    maps = []
    for j in range(N_CORES):
        sl = slice(j * E, (j + 1) * E)
        maps.append({
            "xT": xT,
            "wxcT": np.ascontiguousarray(W_in[sl, :].T),
            "wzT": np.ascontiguousarray(W_in[ED + j * E:ED + (j + 1) * E, :].T),
            "convw": np.ascontiguousarray(np.asarray(conv_w, np.float32)[sl]),
            "convb": np.ascontiguousarray(np.asarray(conv_b, np.float32)[sl])[:, None],
            "wxT": np.ascontiguousarray(np.asarray(W_x, np.float32)[:, sl].T),
        })
    return maps


def _fir_mats():
    """Banded Toeplitz taps for v = EMA_beta(u) truncated at 16 taps.
    W0[t', t] = beta^(t-t') for 0 <= t-t' < 16 (within time block);
    W1[t', t] = beta^(t+128-t') for 1 <= t+128-t' < 16 (carry from prev block).
    """
    J = 16
    w0 = np.zeros((128, 128), np.float32)
    w1 = np.zeros((128, 128), np.float32)
    for tp in range(128):
        for t in range(128):
            d = t - tp
            if 0 <= d < J:
                w0[tp, t] = BETA ** d
            d2 = t + 128 - tp
            if 1 <= d2 < J:
                w1[tp, t] = BETA ** d2
    return w0.astype(ml_dtypes.bfloat16), w1.astype(ml_dtypes.bfloat16)


def _in_maps_b(res_a, x, W_in, W_dt, b_dt, A_log, D, W_out):
    A = -np.exp(np.asarray(A_log, np.float32))
    ident = np.eye(128, dtype=ml_dtypes.bfloat16)
    w0, w1 = _fir_mats()
    dbc = np.zeros((DBC, L), np.float32)
    for j in range(N_CORES):
        dbc += np.asarray(res_a[j]["dbcp_o"], np.float32)
    dbc = dbc.astype(ml_dtypes.bfloat16)
    maps = []
    for j in range(N_CORES):
        sl = slice(j * E, (j + 1) * E)
        maps.append({
            "xc_i": res_a[j]["xc_o"],
            "zs_i": res_a[j]["zs_o"],
            "dbc_i": dbc,
            "wdtT": np.ascontiguousarray(
                np.asarray(W_dt, np.float32)[sl, :].T).astype(ml_dtypes.bfloat16),
            "bdt": np.ascontiguousarray(np.asarray(b_dt, np.float32)[sl])[:, None],
            "acols": np.ascontiguousarray(A[sl, :]),
            "dcol": np.ascontiguousarray(np.asarray(D, np.float32)[sl])[:, None],
            "woutT": np.ascontiguousarray(
                np.asarray(W_out, np.float32)[:, sl].T).astype(ml_dtypes.bfloat16),
            "ident": ident,
            "w0fir": w0,
            "w1fir": w1,
        })
    return maps


def kernel(x, W_in, conv_w, conv_b, W_x, W_dt, b_dt, A_log, D, W_out):
    from concourse.bass_utils import run_bass_kernel_spmd

    nc_a, nc_b = _get_programs()
    res_a = run_bass_kernel_spmd(nc_a, _in_maps_a(x, W_in, conv_w, conv_b, W_x),
                                 list(range(N_CORES))).results
    res_b = run_bass_kernel_spmd(nc_b,
                                 _in_maps_b(res_a, x, W_in, W_dt, b_dt, A_log, D, W_out),
                                 list(range(N_CORES))).results
    out_T = np.zeros((D_MODEL, L), np.float64)
    for j in range(N_CORES):
        out_T += res_b[j]["out_pT"]
    return out_T.T[None, :, :].astype(np.float32)
